# revision 15
# baseline (speedup 1.0000x reference)
"""Trainium2 Bass kernel for nn_MultiHeadAttention (B=8, S=1024, D=768, H=12).

Sharding: data-parallel over batch — one batch element per NeuronCore (8 cores).
No collectives needed; gather is a host-side stack.

bf16 compute with fp32 PSUM accumulation. Per-core layout:
  inputs (host-prepped, bf16): xqT/xkT/xvT (D,S); WqT/WkT (D,D); WvT_pad
  (D, 12*65) with zero columns at each head's slot 64; WoT (D,D); I128
  identity; fp32 biases (bv folded into bo on host: bo_eff = bo + Wo @ bv).
  - QT[do,s] = WqT.T @ xqT + bq ; KT[do,s] = WkT.T @ xkT + bk  (feature-major)
  - V[t,dpad] = xvT.T @ WvT_pad (natural layout, 65-wide head slots with a
    ones column per head so attn@V also yields the softmax denominator)
  - per head pair j (heads 2j at partitions 0:64, 2j+1 at 64:128):
      scoresT[t,s] = KT_h.T @ QT_h   (row-packed K=64 matmul pair)
      E = exp(SCALE * scoresT)       (ScalarE, PSUM->SBUF bf16, both heads)
  - attn@V runs with E stationary (M=128 queries) and V moving (N=65):
      O_sd[s, d+Z] += E_tile.T @ V_aug   — 65-cycle matmuls, fp32 PSUM
    normalize per partition (Z is a column): O_sd[:,0:64] *= 1/Z, then
    PE-transpose the assembled [s,128] pair tile back to feature-major OHT.
  - O[s,do] = OHT.T @ WoT + bo_eff

Schedule: DMAs are issued in deadline order (K/Q pair-0 critical blocks
first) so the exp stream starts ~7us in instead of ~33us. Projections are
per-half generators driven as filler between exp-paced attention steps;
V-projection runs as filler during the first attention units. Each unit's
attn@V batches are deferred wholesale into the following unit (one batch
per interleave point), so V never gates the exp stream.
"""
import sys

sys.path.insert(0, "/opt/trn_rl_repo")

import numpy as np
from ml_dtypes import bfloat16

import concourse.bacc as bacc
import concourse.tile as tile
from concourse import mybir
from concourse.bass_utils import run_bass_kernel_spmd

B, S, D, H = 8, 1024, 768, 12
DH = D // H                       # 64
NP = H // 2                       # 6 head pairs == D/128 tiles
DVP = H * (DH + 1)                # 780: V padded width (65 per head)
SCALE = 1.0 / np.sqrt(np.float32(D))
NT = S // 128                     # 8 seq tiles of 128
ND = D // 128                     # 6 feature tiles of 128

F32 = mybir.dt.float32
BF16 = mybir.dt.bfloat16
F8 = mybir.dt.float8e4
Exp = mybir.ActivationFunctionType.Exp

_CACHE = {}


def _build_nc(debug_outputs=False, loop_n=1):
    nc = bacc.Bacc("TRN2", target_bir_lowering=False, debug=False)

    d = {}
    for name, shape in [
        ("xqt", (D, S)), ("xkt", (D, S)), ("xvt", (D, S)),
        ("wqt", (D, D)), ("wkt", (D, D)), ("wvtp", (D, DVP)), ("wot", (D, D)),
        ("ident", (128, 128)),
    ]:
        d[name] = nc.dram_tensor(name, shape, BF16, kind="ExternalInput").ap()
    for name, shape in [("bqc", (128, ND)), ("bkc", (128, ND)),
                        ("bor", (1, D))]:
        d[name] = nc.dram_tensor(name, shape, F32, kind="ExternalInput").ap()
    out_d = nc.dram_tensor("out", (S, D), BF16, kind="ExternalOutput").ap()

    with tile.TileContext(nc) as tc:
        for _ in range(loop_n):
            _emit(nc, tc, d, out_d)
    nc.compile()
    return nc


def _emit(nc, tc, d, out_d):
    import contextlib

    ctx = contextlib.ExitStack()
    with ctx:
        w_pool = ctx.enter_context(tc.tile_pool(name="w", bufs=24))
        x_pool = ctx.enter_context(tc.tile_pool(name="x", bufs=18))
        qk_pool = ctx.enter_context(tc.tile_pool(name="qk", bufs=12))
        v_pool = ctx.enter_context(tc.tile_pool(name="v", bufs=8))
        e_pool = ctx.enter_context(tc.tile_pool(name="e", bufs=16))
        oht_pool = ctx.enter_context(tc.tile_pool(name="oht", bufs=6))
        o_pool = ctx.enter_context(tc.tile_pool(name="o", bufs=2))
        osd_pool = ctx.enter_context(tc.tile_pool(name="osd", bufs=6))
        rz_pool = ctx.enter_context(tc.tile_pool(name="rz", bufs=4))
        const_pool = ctx.enter_context(tc.tile_pool(name="const", bufs=1))
        ps = ctx.enter_context(tc.tile_pool(name="ps", bufs=2, space="PSUM"))
        ps_acc = ctx.enter_context(
            tc.tile_pool(name="ps_acc", bufs=2, space="PSUM"))
        ps_proj = ctx.enter_context(
            tc.tile_pool(name="ps_proj", bufs=2, space="PSUM"))

        # ---- constants ----
        bq_t = const_pool.tile([128, ND], F32, name="bq_t")
        bk_t = const_pool.tile([128, ND], F32, name="bk_t")
        bo_bc = const_pool.tile([128, D], F32, name="bo_bc")
        ident = const_pool.tile([128, 128], BF16, name="ident")

        qs = (nc.sync, nc.scalar, nc.gpsimd)

        # ---- staged input tiles ----
        wq = [w_pool.tile([128, D], BF16, name=f"wqt{i}", tag="w")
              for i in range(ND)]
        wk = [w_pool.tile([128, D], BF16, name=f"wkt{i}", tag="w")
              for i in range(ND)]
        wv = [w_pool.tile([128, DVP], BF16, name=f"wvtp{i}", tag="w")
              for i in range(ND)]
        xq = [x_pool.tile([128, S], BF16, name=f"xqt{i}", tag="x")
              for i in range(ND)]
        xk = [x_pool.tile([128, S], BF16, name=f"xkt{i}", tag="x")
              for i in range(ND)]
        xv = [x_pool.tile([128, S], BF16, name=f"xvt{i}", tag="x")
              for i in range(ND)]

        def dma_cols(tiles, key, c0, c1, queues):
            for i in range(ND):
                queues[i % len(queues)].dma_start(
                    tiles[i][:, c0:c1], d[key][i * 128:(i + 1) * 128, c0:c1])

        # ---- DMA issue in deadline order (~60 DMAs, ~baseline count).
        # The scalar (ACT) queue carries almost no DMA issue before/during
        # the early exp stream so exps are never delayed at the SEQ. ----
        nc.gpsimd.dma_start(bq_t[:], d["bqc"][:])
        nc.gpsimd.dma_start(bk_t[:], d["bkc"][:])
        dma_cols(wk, "wkt", 0, 128, qs)    # pair-0 weight blocks (tiny)
        dma_cols(wq, "wqt", 0, 128, qs)
        # pre-warm the exp table set; only 2 tiny DMAs precede this on the
        # ACT queue so the table load lands well before the first scores
        warm = const_pool.tile([128, ND], BF16, name="warm")
        nc.scalar.activation(warm[:], bq_t[:], Exp)
        sg = (nc.sync, nc.gpsimd)
        # interleave the K/Q activation strips per-di so the pair-0
        # projections pipeline right behind the arriving stripes
        for i in range(ND):
            sg[i % 2].dma_start(xk[i][:, 0:512],
                                d["xkt"][i * 128:(i + 1) * 128, 0:512])
            sg[(i + 1) % 2].dma_start(xq[i][:, 0:512],
                                      d["xqt"][i * 128:(i + 1) * 128, 0:512])
        dma_cols(xk, "xkt", 512, 1024, sg)  # keys strip 1 (scores tb>=4)
        nc.gpsimd.dma_start(ident[:], d["ident"][:])
        dma_cols(wq, "wqt", 128, 768, qs)  # pairs 1..5 weight blocks
        dma_cols(wk, "wkt", 128, 768, qs)
        dma_cols(wv, "wvtp", 0, DVP, sg)   # V weights
        dma_cols(xv, "xvt", 0, S, sg)      # values, full stripes
        dma_cols(xq, "xqt", 512, 1024, sg)  # queries strip 1
        nc.gpsimd.dma_start(bo_bc[:], d["bor"].to_broadcast((128, D)))

        # ---- generator driver with completion keys ----
        active = []          # [key, generator] in deadline order
        done = set()

        def spawn(key, gen):
            active.append((key, gen))

        def drive(n):
            while n > 0 and active:
                try:
                    next(active[0][1])
                    n -= 1
                except StopIteration:
                    done.add(active.pop(0)[0])

        def drive_all():
            while active:
                drive(64)

        def drive_until(key):
            while key not in done and active:
                try:
                    next(active[0][1])
                except StopIteration:
                    done.add(active.pop(0)[0])

        # ---- projections as per-half generators ----
        qt_tiles, kt_tiles = {}, {}

        def gen_qk_half(which, w_t, x_t, b_t, p, hh, ot):
            pp = ps_proj.tile([128, 512], F32, name=f"{which}pp{p}_{hh}",
                              tag="pp")
            for di in range(ND):
                nc.tensor.matmul(
                    pp[:], w_t[di][:, p * 128:(p + 1) * 128],
                    x_t[di][:, hh * 512:(hh + 1) * 512],
                    start=di == 0, stop=di == ND - 1)
                yield
            # eviction on DVE: ScalarE stays a pure exp stream
            nc.vector.tensor_scalar_add(
                ot[:, hh * 512:(hh + 1) * 512], pp[:], b_t[:, p:p + 1])

        def spawn_qk_half(which, p, hh):
            w_t, x_t, b_t, tiles = ((wq, xq, bq_t, qt_tiles) if which == "q"
                                    else (wk, xk, bk_t, kt_tiles))
            if p not in tiles:
                tiles[p] = qk_pool.tile([128, S], BF16, name=f"{which}t{p}",
                                        tag="qk")
            spawn(f"{which}{p}h{hh}",
                  gen_qk_half(which, w_t, x_t, b_t, p, hh, tiles[p]))

        # ---- V projection as half generators (one ps_proj buf each) ----
        v_tiles = {}

        def gen_vproj(tb):
            pa = ps_proj.tile([128, 512], F32, name=f"vpa{tb}", tag="pp")
            for di in range(ND):
                nc.tensor.matmul(pa[:], xv[di][:, tb * 128:(tb + 1) * 128],
                                 wv[di][:, 0:512],
                                 start=di == 0, stop=di == ND - 1)
                yield
            vt = v_pool.tile([128, DVP], BF16, name=f"v{tb}", tag="v")
            nc.vector.tensor_copy(vt[:, 0:512], pa[:])
            pb = ps_proj.tile([128, DVP - 512], F32, name=f"vpb{tb}",
                              tag="pp")
            for di in range(ND):
                nc.tensor.matmul(pb[:], xv[di][:, tb * 128:(tb + 1) * 128],
                                 wv[di][:, 512:DVP],
                                 start=di == 0, stop=di == ND - 1)
                yield
            nc.vector.tensor_copy(vt[:, 512:DVP], pb[:])
            # ones columns (head slot 64) for the denominator trick
            v3 = vt[:].rearrange("p (h e) -> p h e", e=DH + 1)
            nc.vector.memset(v3[:, :, DH:DH + 1], 1.0)
            v_tiles[tb] = vt

        # ---- attention steps ----
        st_ctx = {}

        def att_state(p, strip):
            return st_ctx.setdefault((p, strip), {"et": {}, "grp": None,
                                                  "osd": None})

        def att_step(p, strip, tb):
            s = att_state(p, strip)
            sl = slice(strip * 512, strip * 512 + 512)
            sc = ps.tile([128, 1024], F32, name=f"sc{p}_{strip}_{tb}",
                         tag="ps")
            tsl = slice(tb * 128, (tb + 1) * 128)
            nc.tensor.matmul(sc[:, 0:512], kt_tiles[p][0:64, tsl],
                             qt_tiles[p][0:64, sl], start=True, stop=True)
            nc.tensor.matmul(sc[:, 512:1024], kt_tiles[p][64:128, tsl],
                             qt_tiles[p][64:128, sl], start=True, stop=True)
            et = e_pool.tile([128, 1024], BF16, name=f"e{p}_{strip}_{tb}",
                             tag="e")
            nc.scalar.activation(et[:], sc[:], Exp, scale=float(SCALE))
            s["et"][tb] = et

        def att_dmm(p, strip, tb):
            s = att_state(p, strip)
            if s["grp"] is None:
                s["grp"] = [ps_acc.tile([128, 260], F32,
                                        name=f"g{h}_{p}_{strip}", tag="acc")
                            for h in (0, 1)]
            c0 = p * 2 * (DH + 1)
            et = s["et"].pop(tb)
            vt = v_tiles[tb]
            for h in (0, 1):
                vsl = vt[:, c0 + h * (DH + 1):c0 + (h + 1) * (DH + 1)]
                for j in range(4):
                    nc.tensor.matmul(
                        s["grp"][h][:, j * 65:(j + 1) * 65],
                        et[:, h * 512 + j * 128:h * 512 + (j + 1) * 128],
                        vsl, start=tb == 0 and j == 0,
                        stop=tb == NT - 1 and j == 3,
                        skip_group_check=True)

        def att_norm(p, strip):
            s = att_state(p, strip)
            osd = osd_pool.tile([128, 512], BF16, name=f"osd{p}{strip}",
                                tag="osd")
            osd3 = osd[:].rearrange("p (j q) -> p j q", q=128)
            for h in (0, 1):
                rz = rz_pool.tile([128, 4], F32, name=f"rz{p}{strip}{h}",
                                  tag="rz")
                nc.vector.reciprocal_approx_fast(
                    rz[:], s["grp"][h][:, DH::DH + 1])
                g3 = s["grp"][h][:].rearrange("p (j e) -> p j e", e=DH + 1)
                nc.vector.tensor_mul(
                    osd3[:, :, h * DH:(h + 1) * DH], g3[:, :, 0:DH],
                    rz[:].unsqueeze(-1).broadcast_to((128, 4, DH)))
            s["osd"] = osd

        def att_transpose(p, strip):
            s = st_ctx.pop((p, strip))
            sl = slice(strip * 512, strip * 512 + 512)
            tr = ps_proj.tile([128, 512], BF16, name=f"tr{p}_{strip}",
                              tag="pp")
            for j in range(4):
                nc.tensor.transpose(tr[:, j * 128:(j + 1) * 128],
                                    s["osd"][:, j * 128:(j + 1) * 128],
                                    ident[:])
            nc.vector.tensor_copy(oht_tiles[p][:, sl], tr[:])

        oht_tiles = [
            oht_pool.tile([128, S], BF16, name=f"oht{p}", tag="oht")
            for p in range(NP)
        ]
        wo = []

        def gen_oproj(stt):
            ssl = slice(stt * 128, (stt + 1) * 128)
            o_t = o_pool.tile([128, D], BF16, name=f"o{stt}", tag="o")
            for hh, w in ((0, 512), (1, 256)):
                pp = ps_proj.tile([128, w], F32, name=f"opp{stt}_{hh}",
                                  tag="pp")
                csl = slice(hh * 512, hh * 512 + w)
                for di in range(ND):
                    nc.tensor.matmul(pp[:], oht_tiles[di][:, ssl],
                                     wo[di][:, csl],
                                     start=di == 0, stop=di == ND - 1)
                    yield
                nc.vector.tensor_add(o_t[:, csl], pp[:], bo_bc[:, csl])
                nc.sync.dma_start(out_d[ssl, csl], o_t[:, csl])

        def spawn_filler(p, strip):
            # projections spawn TWO units ahead of use so unit boundaries
            # never block in drive_until
            if strip == 0:
                if p == 0:
                    # interleave the V-proj units with the pair-2 trio so
                    # both complete within unit 1 (batches force the vps;
                    # the trio rides along before vp7)
                    for tb in range(NT):
                        spawn(f"vp{tb}", gen_vproj(tb))
                        if tb in (1, 3, 5):
                            spawn_qk_half(("q", "k", "k")[tb // 2], 2,
                                          (0, 0, 1)[tb // 2])
                elif p + 2 < NP:
                    spawn_qk_half("q", p + 2, 0)
                    spawn_qk_half("k", p + 2, 0)
                    spawn_qk_half("k", p + 2, 1)
                if p == 4:
                    spawn_qk_half("q", 0, 1)
                if p == 5:
                    spawn_qk_half("q", 1, 1)
                    for i in range(ND):
                        t = w_pool.tile([128, D], BF16, name=f"wot{i}",
                                        tag="w")
                        nc.sync.dma_start(
                            t[:], d["wot"][i * 128:(i + 1) * 128, :])
                        wo.append(t)
            else:
                if p + 2 < NP:
                    spawn_qk_half("q", p + 2, 1)
                # o-proj spawns trail the transpose pipeline by one unit:
                # oproj(k) reads all six strip-0 OHT tiles, the last of
                # which ((5,0)) is only transposed at unit 8's head
                if 2 <= p <= 5:
                    spawn(f"op{p - 2}", gen_oproj(p - 2))

        # ---- prologue: pair-0 h0 projections, then attention ----
        spawn_qk_half("k", 0, 0)
        spawn_qk_half("q", 0, 0)
        drive_all()
        spawn_qk_half("k", 0, 1)
        spawn_qk_half("q", 1, 0)
        spawn_qk_half("k", 1, 0)
        spawn_qk_half("k", 1, 1)

        # ---- exp-paced attention with a lagged dmm-batch FIFO: a unit's
        # attn@V batches drain during the following ~1.5 units, matching
        # when the V tiles' DMA+projection can actually complete ----
        seq = [(p, 0) for p in range(NP)] + [(p, 1) for p in range(NP)]
        pending = []          # FIFO of [p, strip, remaining tbs]
        tr_backlog = []       # normalized units awaiting PE transpose

        def pend_batch():
            if not pending:
                return
            ent = pending[0]
            tb = ent[2].pop(0)
            drive_until(f"vp{tb}")
            att_dmm(ent[0], ent[1], tb)
            if not ent[2]:
                att_norm(ent[0], ent[1])
                tr_backlog.append((ent[0], ent[1]))
                pending.pop(0)

        for idx, (p, strip) in enumerate(seq):
            drive_until(f"q{p}h{strip}")
            if strip == 0:
                # k{p}h1 is NOT forced here: scores tb<4 only read kt h0;
                # the tb-loop drives pull h1 through before tb=4 needs it
                drive_until(f"k{p}h0")
            att_step(p, strip, 0)
            if idx > 1:
                pend_batch()
            # transposes pop at the unit head so they always precede the
            # o-proj di=5 matmuls in the in-order PE queue
            if tr_backlog:
                att_transpose(*tr_backlog.pop(0))
            att_step(p, strip, 1)
            if idx > 1:
                pend_batch()
            if tr_backlog:
                att_transpose(*tr_backlog.pop(0))
            spawn_filler(p, strip)
            for tb in range(2, NT):
                drive(1)
                att_step(p, strip, tb)
                drive(2)
                if idx > 1 or (idx == 1 and tb >= 6):
                    pend_batch()
            pending.append([p, strip, list(range(NT))])

        # ---- epilogue ----
        drive_all()
        # finish the second-to-last unit's batches + transpose before the
        # tail o-proj generators reference its OHT
        while len(pending) > 1:
            pend_batch()
        while tr_backlog:
            att_transpose(*tr_backlog.pop(0))
        for stt in range(4, 8):
            spawn(f"op{stt}", gen_oproj(stt))
        # 5 safe yields (di 0..4 of first half) fill the final exp wait; a
        # 6th would emit a di=5 matmul ahead of the final transpose in the
        # in-order PE queue and deadlock on it
        drive(5)
        while pending:
            pend_batch()
        while tr_backlog:
            att_transpose(*tr_backlog.pop(0))
        drive_all()


def _prep(queries, keys, values, Wq, bq, Wk, bk, Wv, bv, Wo, bo):
    """Host-side prep: returns per-core input dicts."""
    wvt = np.asarray(Wv, np.float32).T              # (D, D) = (di, do)
    wvtp = np.zeros((D, DVP), np.float32)
    for h in range(H):
        wvtp[:, h * (DH + 1):h * (DH + 1) + DH] = \
            wvt[:, h * DH:(h + 1) * DH]
    bo_eff = (np.asarray(bo, np.float32)
              + np.asarray(Wo, np.float32) @ np.asarray(bv, np.float32))
    bf = lambda a: np.ascontiguousarray(np.asarray(a, np.float32)).astype(
        bfloat16)
    shared = {
        "wqt": bf(np.asarray(Wq, np.float32).T),
        "wkt": bf(np.asarray(Wk, np.float32).T),
        "wvtp": wvtp.astype(bfloat16),
        "wot": bf(np.asarray(Wo, np.float32).T),
        "ident": np.eye(128, dtype=np.float32).astype(bfloat16),
        "bqc": np.ascontiguousarray(
            np.asarray(bq, np.float32).reshape(ND, 128).T),
        "bkc": np.ascontiguousarray(
            np.asarray(bk, np.float32).reshape(ND, 128).T),
        "bor": np.ascontiguousarray(bo_eff.reshape(1, D)),
    }
    queries = np.asarray(queries, np.float32)
    keys = np.asarray(keys, np.float32)
    values = np.asarray(values, np.float32)
    in_maps = []
    for b in range(B):
        in_maps.append({
            "xqt": bf(queries[b].T),
            "xkt": bf(keys[b].T),
            "xvt": bf(values[b].T),
            **shared,
        })
    return in_maps


def _get_nc():
    if "nc" not in _CACHE:
        _CACHE["nc"] = _build_nc()
    return _CACHE["nc"]


def kernel(queries, keys, values, Wq, bq, Wk, bk, Wv, bv, Wo, bo):
    in_maps = _prep(queries, keys, values, Wq, bq, Wk, bk, Wv, bv, Wo, bo)
    nc = _get_nc()
    res = run_bass_kernel_spmd(nc, in_maps, core_ids=list(range(B)))
    return np.stack([res.results[b]["out"].astype(np.float32)
                     for b in range(B)], axis=0)


# revision 16
# speedup vs baseline: 1.0546x; 1.0546x over previous
"""Trainium2 Bass kernel for nn_MultiHeadAttention (B=8, S=1024, D=768, H=12).

Sharding: data-parallel over batch — one batch element per NeuronCore (8 cores).
No collectives needed; gather is a host-side stack.

bf16 compute with fp32 PSUM accumulation. Per-core layout:
  inputs (host-prepped, bf16): xqT/xkT/xvT (D,S); WqT/WkT (D,D); WvT_pad
  (D, 12*65) with zero columns at each head's slot 64; WoT (D,D); I128
  identity; fp32 biases (bv folded into bo on host: bo_eff = bo + Wo @ bv).
  - QT[do,s] = WqT.T @ xqT + bq ; KT[do,s] = WkT.T @ xkT + bk  (feature-major)
  - V[t,dpad] = xvT.T @ WvT_pad (natural layout, 65-wide head slots with a
    ones column per head so attn@V also yields the softmax denominator)
  - per head pair j (heads 2j at partitions 0:64, 2j+1 at 64:128):
      scoresT[t,s] = KT_h.T @ QT_h   (row-packed K=64 matmul pair)
      E = exp(SCALE * scoresT)       (ScalarE, PSUM->SBUF bf16, both heads)
  - attn@V runs with E stationary (M=128 queries) and V moving (N=65):
      O_sd[s, d+Z] += E_tile.T @ V_aug   — 65-cycle matmuls, fp32 PSUM
    normalize per partition (Z is a column): O_sd[:,0:64] *= 1/Z, then
    PE-transpose the assembled [s,128] pair tile back to feature-major OHT.
  - O[s,do] = OHT.T @ WoT + bo_eff

Schedule: DMAs are issued in deadline order (K/Q pair-0 critical blocks
first) so the exp stream starts ~7us in instead of ~33us. Projections are
per-half generators driven as filler between exp-paced attention steps;
V-projection runs as filler during the first attention units. Each unit's
attn@V batches are deferred wholesale into the following unit (one batch
per interleave point), so V never gates the exp stream.
"""
import sys

sys.path.insert(0, "/opt/trn_rl_repo")

import numpy as np
from ml_dtypes import bfloat16

import concourse.bacc as bacc
import concourse.tile as tile
from concourse import mybir
from concourse.bass_utils import run_bass_kernel_spmd

B, S, D, H = 8, 1024, 768, 12
DH = D // H                       # 64
NP = H // 2                       # 6 head pairs == D/128 tiles
DVP = H * (DH + 1)                # 780: V padded width (65 per head)
SCALE = 1.0 / np.sqrt(np.float32(D))
NT = S // 128                     # 8 seq tiles of 128
ND = D // 128                     # 6 feature tiles of 128

F32 = mybir.dt.float32
BF16 = mybir.dt.bfloat16
F8 = mybir.dt.float8e4
Exp = mybir.ActivationFunctionType.Exp

_CACHE = {}


def _build_nc(debug_outputs=False, loop_n=1):
    nc = bacc.Bacc("TRN2", target_bir_lowering=False, debug=False)

    d = {}
    for name, shape in [
        ("xqt", (D, S)), ("xkt", (D, S)), ("xvt", (D, S)),
        ("wqt", (D, D)), ("wkt", (D, D)), ("wvtp", (D, DVP)), ("wot", (D, D)),
        ("ident", (128, 128)),
    ]:
        d[name] = nc.dram_tensor(name, shape, BF16, kind="ExternalInput").ap()
    for name, shape in [("bqc", (128, ND)), ("bkc", (128, ND)),
                        ("bor", (1, D))]:
        d[name] = nc.dram_tensor(name, shape, F32, kind="ExternalInput").ap()
    out_d = nc.dram_tensor("out", (S, D), BF16, kind="ExternalOutput").ap()

    with tile.TileContext(nc) as tc:
        for _ in range(loop_n):
            _emit(nc, tc, d, out_d)
    nc.compile()
    return nc


def _emit(nc, tc, d, out_d):
    import contextlib

    ctx = contextlib.ExitStack()
    with ctx:
        w_pool = ctx.enter_context(tc.tile_pool(name="w", bufs=24))
        x_pool = ctx.enter_context(tc.tile_pool(name="x", bufs=18))
        qk_pool = ctx.enter_context(tc.tile_pool(name="qk", bufs=12))
        v_pool = ctx.enter_context(tc.tile_pool(name="v", bufs=8))
        e_pool = ctx.enter_context(tc.tile_pool(name="e", bufs=16))
        oht_pool = ctx.enter_context(tc.tile_pool(name="oht", bufs=6))
        o_pool = ctx.enter_context(tc.tile_pool(name="o", bufs=2))
        osd_pool = ctx.enter_context(tc.tile_pool(name="osd", bufs=6))
        rz_pool = ctx.enter_context(tc.tile_pool(name="rz", bufs=4))
        const_pool = ctx.enter_context(tc.tile_pool(name="const", bufs=1))
        ps = ctx.enter_context(tc.tile_pool(name="ps", bufs=2, space="PSUM"))
        ps_acc = ctx.enter_context(
            tc.tile_pool(name="ps_acc", bufs=2, space="PSUM"))
        ps_proj = ctx.enter_context(
            tc.tile_pool(name="ps_proj", bufs=2, space="PSUM"))

        # ---- constants ----
        bq_t = const_pool.tile([128, ND], F32, name="bq_t")
        bk_t = const_pool.tile([128, ND], F32, name="bk_t")
        bo_bc = const_pool.tile([128, D], F32, name="bo_bc")
        ident = const_pool.tile([128, 128], BF16, name="ident")

        qs = (nc.sync, nc.scalar, nc.gpsimd)

        # ---- staged input tiles ----
        wq = [w_pool.tile([128, D], BF16, name=f"wqt{i}", tag="w")
              for i in range(ND)]
        wk = [w_pool.tile([128, D], BF16, name=f"wkt{i}", tag="w")
              for i in range(ND)]
        wv = [w_pool.tile([128, DVP], BF16, name=f"wvtp{i}", tag="w")
              for i in range(ND)]
        xq = [x_pool.tile([128, S], BF16, name=f"xqt{i}", tag="x")
              for i in range(ND)]
        xk = [x_pool.tile([128, S], BF16, name=f"xkt{i}", tag="x")
              for i in range(ND)]
        xv = [x_pool.tile([128, S], BF16, name=f"xvt{i}", tag="x")
              for i in range(ND)]

        def dma_cols(tiles, key, c0, c1, queues):
            for i in range(ND):
                queues[i % len(queues)].dma_start(
                    tiles[i][:, c0:c1], d[key][i * 128:(i + 1) * 128, c0:c1])

        # ---- DMA issue in deadline order (53 DMAs). Weights go as FULL
        # stripes (1.5KB/partition lines — sub-512B lines gut real DMA
        # throughput); only the x activations split into 1KB halves. The
        # scalar (ACT) queue carries almost no DMA issue before/during the
        # early exp stream so exps are never delayed at the SEQ. ----
        nc.gpsimd.dma_start(bq_t[:], d["bqc"][:])
        nc.gpsimd.dma_start(bk_t[:], d["bkc"][:])
        dma_cols(wk, "wkt", 0, D, qs)      # K weights, full stripes
        # pre-warm the exp table set; only 2 tiny DMAs precede this on the
        # ACT queue so the table load lands well before the first scores
        warm = const_pool.tile([128, ND], BF16, name="warm")
        nc.scalar.activation(warm[:], bq_t[:], Exp)
        sg = (nc.sync, nc.gpsimd)
        dma_cols(xk, "xkt", 0, 512, sg)    # keys strip 0
        dma_cols(wq, "wqt", 0, D, sg)      # Q weights, full stripes
        dma_cols(xq, "xqt", 0, 512, sg)    # queries strip 0 -> first exp
        dma_cols(xk, "xkt", 512, 1024, sg)  # keys strip 1 (scores tb>=4)
        nc.gpsimd.dma_start(ident[:], d["ident"][:])
        dma_cols(wv, "wvtp", 0, DVP, sg)   # V weights
        dma_cols(xv, "xvt", 0, S, sg)      # values, full stripes
        dma_cols(xq, "xqt", 512, 1024, sg)  # queries strip 1
        nc.gpsimd.dma_start(bo_bc[:], d["bor"].to_broadcast((128, D)))

        # ---- generator driver with completion keys ----
        active = []          # [key, generator] in deadline order
        done = set()

        def spawn(key, gen):
            active.append((key, gen))

        def drive(n):
            while n > 0 and active:
                try:
                    next(active[0][1])
                    n -= 1
                except StopIteration:
                    done.add(active.pop(0)[0])

        def drive_all():
            while active:
                drive(64)

        def drive_until(key):
            while key not in done and active:
                try:
                    next(active[0][1])
                except StopIteration:
                    done.add(active.pop(0)[0])

        # ---- projections as per-half generators ----
        qt_tiles, kt_tiles = {}, {}

        def gen_qk_half(which, w_t, x_t, b_t, p, hh, ot):
            pp = ps_proj.tile([128, 512], F32, name=f"{which}pp{p}_{hh}",
                              tag="pp")
            for di in range(ND):
                nc.tensor.matmul(
                    pp[:], w_t[di][:, p * 128:(p + 1) * 128],
                    x_t[di][:, hh * 512:(hh + 1) * 512],
                    start=di == 0, stop=di == ND - 1)
                yield
            # eviction on DVE: ScalarE stays a pure exp stream
            nc.vector.tensor_scalar_add(
                ot[:, hh * 512:(hh + 1) * 512], pp[:], b_t[:, p:p + 1])

        def spawn_qk_half(which, p, hh):
            w_t, x_t, b_t, tiles = ((wq, xq, bq_t, qt_tiles) if which == "q"
                                    else (wk, xk, bk_t, kt_tiles))
            if p not in tiles:
                tiles[p] = qk_pool.tile([128, S], BF16, name=f"{which}t{p}",
                                        tag="qk")
            spawn(f"{which}{p}h{hh}",
                  gen_qk_half(which, w_t, x_t, b_t, p, hh, tiles[p]))

        # ---- V projection as half generators (one ps_proj buf each) ----
        v_tiles = {}

        def gen_vproj(tb):
            pa = ps_proj.tile([128, 512], F32, name=f"vpa{tb}", tag="pp")
            for di in range(ND):
                nc.tensor.matmul(pa[:], xv[di][:, tb * 128:(tb + 1) * 128],
                                 wv[di][:, 0:512],
                                 start=di == 0, stop=di == ND - 1)
                yield
            vt = v_pool.tile([128, DVP], BF16, name=f"v{tb}", tag="v")
            nc.vector.tensor_copy(vt[:, 0:512], pa[:])
            pb = ps_proj.tile([128, DVP - 512], F32, name=f"vpb{tb}",
                              tag="pp")
            for di in range(ND):
                nc.tensor.matmul(pb[:], xv[di][:, tb * 128:(tb + 1) * 128],
                                 wv[di][:, 512:DVP],
                                 start=di == 0, stop=di == ND - 1)
                yield
            nc.vector.tensor_copy(vt[:, 512:DVP], pb[:])
            # ones columns (head slot 64) for the denominator trick
            v3 = vt[:].rearrange("p (h e) -> p h e", e=DH + 1)
            nc.vector.memset(v3[:, :, DH:DH + 1], 1.0)
            v_tiles[tb] = vt

        # ---- attention steps ----
        st_ctx = {}

        def att_state(p, strip):
            return st_ctx.setdefault((p, strip), {"et": {}, "grp": None,
                                                  "osd": None})

        def att_step(p, strip, tb):
            s = att_state(p, strip)
            sl = slice(strip * 512, strip * 512 + 512)
            sc = ps.tile([128, 1024], F32, name=f"sc{p}_{strip}_{tb}",
                         tag="ps")
            tsl = slice(tb * 128, (tb + 1) * 128)
            nc.tensor.matmul(sc[:, 0:512], kt_tiles[p][0:64, tsl],
                             qt_tiles[p][0:64, sl], start=True, stop=True)
            nc.tensor.matmul(sc[:, 512:1024], kt_tiles[p][64:128, tsl],
                             qt_tiles[p][64:128, sl], start=True, stop=True)
            et = e_pool.tile([128, 1024], BF16, name=f"e{p}_{strip}_{tb}",
                             tag="e")
            nc.scalar.activation(et[:], sc[:], Exp, scale=float(SCALE))
            s["et"][tb] = et

        def att_dmm(p, strip, tb):
            s = att_state(p, strip)
            if s["grp"] is None:
                s["grp"] = [ps_acc.tile([128, 260], F32,
                                        name=f"g{h}_{p}_{strip}", tag="acc")
                            for h in (0, 1)]
            c0 = p * 2 * (DH + 1)
            et = s["et"].pop(tb)
            vt = v_tiles[tb]
            for h in (0, 1):
                vsl = vt[:, c0 + h * (DH + 1):c0 + (h + 1) * (DH + 1)]
                for j in range(4):
                    nc.tensor.matmul(
                        s["grp"][h][:, j * 65:(j + 1) * 65],
                        et[:, h * 512 + j * 128:h * 512 + (j + 1) * 128],
                        vsl, start=tb == 0 and j == 0,
                        stop=tb == NT - 1 and j == 3,
                        skip_group_check=True)

        def att_norm(p, strip):
            s = att_state(p, strip)
            osd = osd_pool.tile([128, 512], BF16, name=f"osd{p}{strip}",
                                tag="osd")
            osd3 = osd[:].rearrange("p (j q) -> p j q", q=128)
            for h in (0, 1):
                rz = rz_pool.tile([128, 4], F32, name=f"rz{p}{strip}{h}",
                                  tag="rz")
                nc.vector.reciprocal_approx_fast(
                    rz[:], s["grp"][h][:, DH::DH + 1])
                g3 = s["grp"][h][:].rearrange("p (j e) -> p j e", e=DH + 1)
                nc.vector.tensor_mul(
                    osd3[:, :, h * DH:(h + 1) * DH], g3[:, :, 0:DH],
                    rz[:].unsqueeze(-1).broadcast_to((128, 4, DH)))
            s["osd"] = osd

        def att_transpose(p, strip):
            s = st_ctx.pop((p, strip))
            sl = slice(strip * 512, strip * 512 + 512)
            tr = ps_proj.tile([128, 512], BF16, name=f"tr{p}_{strip}",
                              tag="pp")
            for j in range(4):
                nc.tensor.transpose(tr[:, j * 128:(j + 1) * 128],
                                    s["osd"][:, j * 128:(j + 1) * 128],
                                    ident[:])
            nc.vector.tensor_copy(oht_tiles[p][:, sl], tr[:])

        oht_tiles = [
            oht_pool.tile([128, S], BF16, name=f"oht{p}", tag="oht")
            for p in range(NP)
        ]
        wo = []

        def gen_oproj(stt):
            ssl = slice(stt * 128, (stt + 1) * 128)
            o_t = o_pool.tile([128, D], BF16, name=f"o{stt}", tag="o")
            for hh, w in ((0, 512), (1, 256)):
                pp = ps_proj.tile([128, w], F32, name=f"opp{stt}_{hh}",
                                  tag="pp")
                csl = slice(hh * 512, hh * 512 + w)
                for di in range(ND):
                    nc.tensor.matmul(pp[:], oht_tiles[di][:, ssl],
                                     wo[di][:, csl],
                                     start=di == 0, stop=di == ND - 1)
                    yield
                nc.vector.tensor_add(o_t[:, csl], pp[:], bo_bc[:, csl])
                nc.sync.dma_start(out_d[ssl, csl], o_t[:, csl])

        def spawn_filler(p, strip):
            # projections spawn TWO units ahead of use so unit boundaries
            # never block in drive_until
            if strip == 0:
                if p == 0:
                    # interleave the V-proj units with the pair-2 trio so
                    # both complete within unit 1 (batches force the vps;
                    # the trio rides along before vp7)
                    for tb in range(NT):
                        spawn(f"vp{tb}", gen_vproj(tb))
                        if tb in (1, 3, 5):
                            spawn_qk_half(("q", "k", "k")[tb // 2], 2,
                                          (0, 0, 1)[tb // 2])
                elif p + 2 < NP:
                    spawn_qk_half("q", p + 2, 0)
                    spawn_qk_half("k", p + 2, 0)
                    spawn_qk_half("k", p + 2, 1)
                if p == 4:
                    spawn_qk_half("q", 0, 1)
                if p == 5:
                    spawn_qk_half("q", 1, 1)
                    for i in range(ND):
                        t = w_pool.tile([128, D], BF16, name=f"wot{i}",
                                        tag="w")
                        nc.sync.dma_start(
                            t[:], d["wot"][i * 128:(i + 1) * 128, :])
                        wo.append(t)
            else:
                if p + 2 < NP:
                    spawn_qk_half("q", p + 2, 1)
                # o-proj spawns trail the transpose pipeline by one unit:
                # oproj(k) reads all six strip-0 OHT tiles, the last of
                # which ((5,0)) is only transposed at unit 8's head
                if 2 <= p <= 5:
                    spawn(f"op{p - 2}", gen_oproj(p - 2))

        # ---- prologue: pair-0 h0 projections, then attention ----
        spawn_qk_half("k", 0, 0)
        spawn_qk_half("q", 0, 0)
        drive_all()
        spawn_qk_half("k", 0, 1)
        spawn_qk_half("q", 1, 0)
        spawn_qk_half("k", 1, 0)
        spawn_qk_half("k", 1, 1)

        # ---- exp-paced attention with a lagged dmm-batch FIFO: a unit's
        # attn@V batches drain during the following ~1.5 units, matching
        # when the V tiles' DMA+projection can actually complete ----
        seq = [(p, 0) for p in range(NP)] + [(p, 1) for p in range(NP)]
        pending = []          # FIFO of [p, strip, remaining tbs]
        tr_backlog = []       # normalized units awaiting PE transpose

        def pend_batch():
            if not pending:
                return
            ent = pending[0]
            tb = ent[2].pop(0)
            drive_until(f"vp{tb}")
            att_dmm(ent[0], ent[1], tb)
            if not ent[2]:
                att_norm(ent[0], ent[1])
                tr_backlog.append((ent[0], ent[1]))
                pending.pop(0)

        for idx, (p, strip) in enumerate(seq):
            drive_until(f"q{p}h{strip}")
            if strip == 0:
                # k{p}h1 is NOT forced here: scores tb<4 only read kt h0;
                # the tb-loop drives pull h1 through before tb=4 needs it
                drive_until(f"k{p}h0")
            att_step(p, strip, 0)
            if idx > 1:
                pend_batch()
            # transposes pop at the unit head so they always precede the
            # o-proj di=5 matmuls in the in-order PE queue
            if tr_backlog:
                att_transpose(*tr_backlog.pop(0))
            att_step(p, strip, 1)
            if idx > 1:
                pend_batch()
            if tr_backlog:
                att_transpose(*tr_backlog.pop(0))
            spawn_filler(p, strip)
            for tb in range(2, NT):
                drive(1)
                att_step(p, strip, tb)
                drive(2)
                if idx > 1 or (idx == 1 and tb >= 6):
                    pend_batch()
            pending.append([p, strip, list(range(NT))])

        # ---- epilogue ----
        drive_all()
        # finish the second-to-last unit's batches + transpose before the
        # tail o-proj generators reference its OHT
        while len(pending) > 1:
            pend_batch()
        while tr_backlog:
            att_transpose(*tr_backlog.pop(0))
        for stt in range(4, 8):
            spawn(f"op{stt}", gen_oproj(stt))
        # 5 safe yields (di 0..4 of first half) fill the final exp wait; a
        # 6th would emit a di=5 matmul ahead of the final transpose in the
        # in-order PE queue and deadlock on it
        drive(5)
        while pending:
            pend_batch()
        while tr_backlog:
            att_transpose(*tr_backlog.pop(0))
        drive_all()


def _prep(queries, keys, values, Wq, bq, Wk, bk, Wv, bv, Wo, bo):
    """Host-side prep: returns per-core input dicts."""
    wvt = np.asarray(Wv, np.float32).T              # (D, D) = (di, do)
    wvtp = np.zeros((D, DVP), np.float32)
    for h in range(H):
        wvtp[:, h * (DH + 1):h * (DH + 1) + DH] = \
            wvt[:, h * DH:(h + 1) * DH]
    bo_eff = (np.asarray(bo, np.float32)
              + np.asarray(Wo, np.float32) @ np.asarray(bv, np.float32))
    bf = lambda a: np.ascontiguousarray(np.asarray(a, np.float32)).astype(
        bfloat16)
    shared = {
        "wqt": bf(np.asarray(Wq, np.float32).T),
        "wkt": bf(np.asarray(Wk, np.float32).T),
        "wvtp": wvtp.astype(bfloat16),
        "wot": bf(np.asarray(Wo, np.float32).T),
        "ident": np.eye(128, dtype=np.float32).astype(bfloat16),
        "bqc": np.ascontiguousarray(
            np.asarray(bq, np.float32).reshape(ND, 128).T),
        "bkc": np.ascontiguousarray(
            np.asarray(bk, np.float32).reshape(ND, 128).T),
        "bor": np.ascontiguousarray(bo_eff.reshape(1, D)),
    }
    queries = np.asarray(queries, np.float32)
    keys = np.asarray(keys, np.float32)
    values = np.asarray(values, np.float32)
    in_maps = []
    for b in range(B):
        in_maps.append({
            "xqt": bf(queries[b].T),
            "xkt": bf(keys[b].T),
            "xvt": bf(values[b].T),
            **shared,
        })
    return in_maps


def _get_nc():
    if "nc" not in _CACHE:
        _CACHE["nc"] = _build_nc()
    return _CACHE["nc"]


def kernel(queries, keys, values, Wq, bq, Wk, bk, Wv, bv, Wo, bo):
    in_maps = _prep(queries, keys, values, Wq, bq, Wk, bk, Wv, bv, Wo, bo)
    nc = _get_nc()
    res = run_bass_kernel_spmd(nc, in_maps, core_ids=list(range(B)))
    return np.stack([res.results[b]["out"].astype(np.float32)
                     for b in range(B)], axis=0)


# revision 18
# speedup vs baseline: 1.0820x; 1.0260x over previous
"""Trainium2 Bass kernel for nn_MultiHeadAttention (B=8, S=1024, D=768, H=12).

Sharding: data-parallel over batch — one batch element per NeuronCore (8 cores).
No collectives needed; gather is a host-side stack.

bf16 compute with fp32 PSUM accumulation. Per-core layout:
  inputs (host-prepped, bf16): xqT/xkT/xvT (D,S); WqT/WkT (D,D); WvT_pad
  (D, 12*65) with zero columns at each head's slot 64; WoT (D,D); I128
  identity; fp32 biases (bv folded into bo on host: bo_eff = bo + Wo @ bv).
  - QT[do,s] = WqT.T @ xqT + bq ; KT[do,s] = WkT.T @ xkT + bk  (feature-major)
  - V[t,dpad] = xvT.T @ WvT_pad (natural layout, 65-wide head slots with a
    ones column per head so attn@V also yields the softmax denominator)
  - per head pair j (heads 2j at partitions 0:64, 2j+1 at 64:128):
      scoresT[t,s] = KT_h.T @ QT_h   (row-packed K=64 matmul pair)
      E = exp(SCALE * scoresT)       (ScalarE, PSUM->SBUF bf16, both heads)
  - attn@V runs with E stationary (M=128 queries) and V moving (N=65):
      O_sd[s, d+Z] += E_tile.T @ V_aug   — 65-cycle matmuls, fp32 PSUM
    normalize per partition (Z is a column): O_sd[:,0:64] *= 1/Z, then
    PE-transpose the assembled [s,128] pair tile back to feature-major OHT.
  - O[s,do] = OHT.T @ WoT + bo_eff

Schedule: DMAs are issued in deadline order (K/Q pair-0 critical blocks
first) so the exp stream starts ~7us in instead of ~33us. Projections are
per-half generators driven as filler between exp-paced attention steps;
V-projection runs as filler during the first attention units. Each unit's
attn@V batches are deferred wholesale into the following unit (one batch
per interleave point), so V never gates the exp stream.
"""
import sys

sys.path.insert(0, "/opt/trn_rl_repo")

import numpy as np
from ml_dtypes import bfloat16

import concourse.bacc as bacc
import concourse.tile as tile
from concourse import mybir
from concourse.bass_utils import run_bass_kernel_spmd

B, S, D, H = 8, 1024, 768, 12
DH = D // H                       # 64
NP = H // 2                       # 6 head pairs == D/128 tiles
DVP = H * (DH + 1)                # 780: V padded width (65 per head)
SCALE = 1.0 / np.sqrt(np.float32(D))
NT = S // 128                     # 8 seq tiles of 128
ND = D // 128                     # 6 feature tiles of 128

F32 = mybir.dt.float32
BF16 = mybir.dt.bfloat16
F8 = mybir.dt.float8e4
Exp = mybir.ActivationFunctionType.Exp

_CACHE = {}


def _build_nc(debug_outputs=False, loop_n=1):
    nc = bacc.Bacc("TRN2", target_bir_lowering=False, debug=False)

    d = {}
    for name, shape in [
        ("xqt", (D, S)), ("xkt", (D, S)), ("xvt", (D, S)),
        ("wqt", (D, D)), ("wkt", (D, D)), ("wvtp", (D, DVP)), ("wot", (D, D)),
        ("ident", (128, 128)),
    ]:
        d[name] = nc.dram_tensor(name, shape, BF16, kind="ExternalInput").ap()
    for name, shape in [("bqc", (128, ND)), ("bkc", (128, ND)),
                        ("bor", (1, D))]:
        d[name] = nc.dram_tensor(name, shape, F32, kind="ExternalInput").ap()
    out_d = nc.dram_tensor("out", (S, D), BF16, kind="ExternalOutput").ap()

    with tile.TileContext(nc) as tc:
        for _ in range(loop_n):
            _emit(nc, tc, d, out_d)
    nc.compile()
    return nc


def _emit(nc, tc, d, out_d):
    import contextlib

    ctx = contextlib.ExitStack()
    with ctx:
        w_pool = ctx.enter_context(tc.tile_pool(name="w", bufs=24))
        x_pool = ctx.enter_context(tc.tile_pool(name="x", bufs=18))
        qk_pool = ctx.enter_context(tc.tile_pool(name="qk", bufs=12))
        v_pool = ctx.enter_context(tc.tile_pool(name="v", bufs=8))
        e_pool = ctx.enter_context(tc.tile_pool(name="e", bufs=16))
        oht_pool = ctx.enter_context(tc.tile_pool(name="oht", bufs=6))
        o_pool = ctx.enter_context(tc.tile_pool(name="o", bufs=2))
        osd_pool = ctx.enter_context(tc.tile_pool(name="osd", bufs=6))
        rz_pool = ctx.enter_context(tc.tile_pool(name="rz", bufs=4))
        const_pool = ctx.enter_context(tc.tile_pool(name="const", bufs=1))
        ps = ctx.enter_context(tc.tile_pool(name="ps", bufs=2, space="PSUM"))
        ps_acc = ctx.enter_context(
            tc.tile_pool(name="ps_acc", bufs=2, space="PSUM"))
        ps_proj = ctx.enter_context(
            tc.tile_pool(name="ps_proj", bufs=2, space="PSUM"))

        # ---- constants ----
        bq_t = const_pool.tile([128, ND], F32, name="bq_t")
        bk_t = const_pool.tile([128, ND], F32, name="bk_t")
        bo_bc = const_pool.tile([128, D], F32, name="bo_bc")
        ident = const_pool.tile([128, 128], BF16, name="ident")

        qs = (nc.sync, nc.scalar, nc.gpsimd)

        # ---- staged input tiles ----
        wq = [w_pool.tile([128, D], BF16, name=f"wqt{i}", tag="w")
              for i in range(ND)]
        wk = [w_pool.tile([128, D], BF16, name=f"wkt{i}", tag="w")
              for i in range(ND)]
        wv = [w_pool.tile([128, DVP], BF16, name=f"wvtp{i}", tag="w")
              for i in range(ND)]
        xq = [x_pool.tile([128, S], BF16, name=f"xqt{i}", tag="x")
              for i in range(ND)]
        xk = [x_pool.tile([128, S], BF16, name=f"xkt{i}", tag="x")
              for i in range(ND)]
        xv = [x_pool.tile([128, S], BF16, name=f"xvt{i}", tag="x")
              for i in range(ND)]

        def dma_cols(tiles, key, c0, c1, queues):
            for i in range(ND):
                queues[i % len(queues)].dma_start(
                    tiles[i][:, c0:c1], d[key][i * 128:(i + 1) * 128, c0:c1])

        # ---- DMA issue in deadline order (53 DMAs). Weights go as FULL
        # stripes (1.5KB/partition lines — sub-512B lines gut real DMA
        # throughput); only the x activations split into 1KB halves. The
        # scalar (ACT) queue carries almost no DMA issue before/during the
        # early exp stream so exps are never delayed at the SEQ. ----
        nc.gpsimd.dma_start(bq_t[:], d["bqc"][:])
        nc.gpsimd.dma_start(bk_t[:], d["bkc"][:])
        dma_cols(wk, "wkt", 0, D, qs)      # K weights, full stripes
        # pre-warm the exp table set; only 2 tiny DMAs precede this on the
        # ACT queue so the table load lands well before the first scores
        warm = const_pool.tile([128, ND], BF16, name="warm")
        nc.scalar.activation(warm[:], bq_t[:], Exp)
        sg = (nc.sync, nc.gpsimd)
        dma_cols(xk, "xkt", 0, 512, sg)    # keys strip 0
        dma_cols(wq, "wqt", 0, D, sg)      # Q weights, full stripes
        dma_cols(xq, "xqt", 0, 512, sg)    # queries strip 0 -> first exp
        dma_cols(xk, "xkt", 512, 1024, sg)  # keys strip 1 (scores tb>=4)
        nc.gpsimd.dma_start(ident[:], d["ident"][:])
        dma_cols(wv, "wvtp", 0, DVP, sg)   # V weights
        dma_cols(xv, "xvt", 0, S, sg)      # values, full stripes
        dma_cols(xq, "xqt", 512, 1024, sg)  # queries strip 1
        nc.gpsimd.dma_start(bo_bc[:], d["bor"].to_broadcast((128, D)))

        # ---- generator driver with completion keys ----
        active = []          # [key, generator] in deadline order
        done = set()

        def spawn(key, gen):
            active.append((key, gen))

        def drive(n):
            while n > 0 and active:
                try:
                    next(active[0][1])
                    n -= 1
                except StopIteration:
                    done.add(active.pop(0)[0])

        def drive_all():
            while active:
                drive(64)

        def drive_until(key):
            while key not in done and active:
                try:
                    next(active[0][1])
                except StopIteration:
                    done.add(active.pop(0)[0])

        # ---- projections as per-half generators ----
        qt_tiles, kt_tiles = {}, {}

        def gen_qk_half(which, w_t, x_t, b_t, p, hh, ot):
            pp = ps_proj.tile([128, 512], F32, name=f"{which}pp{p}_{hh}",
                              tag="pp")
            for di in range(ND):
                nc.tensor.matmul(
                    pp[:], w_t[di][:, p * 128:(p + 1) * 128],
                    x_t[di][:, hh * 512:(hh + 1) * 512],
                    start=di == 0, stop=di == ND - 1)
                yield
            # eviction on DVE: ScalarE stays a pure exp stream
            nc.vector.tensor_scalar_add(
                ot[:, hh * 512:(hh + 1) * 512], pp[:], b_t[:, p:p + 1])

        def spawn_qk_half(which, p, hh):
            w_t, x_t, b_t, tiles = ((wq, xq, bq_t, qt_tiles) if which == "q"
                                    else (wk, xk, bk_t, kt_tiles))
            if p not in tiles:
                tiles[p] = qk_pool.tile([128, S], BF16, name=f"{which}t{p}",
                                        tag="qk")
            spawn(f"{which}{p}h{hh}",
                  gen_qk_half(which, w_t, x_t, b_t, p, hh, tiles[p]))

        # ---- V projection as half generators (one ps_proj buf each) ----
        v_tiles = {}

        def gen_vproj(tb):
            pa = ps_proj.tile([128, 512], F32, name=f"vpa{tb}", tag="pp")
            for di in range(ND):
                nc.tensor.matmul(pa[:], xv[di][:, tb * 128:(tb + 1) * 128],
                                 wv[di][:, 0:512],
                                 start=di == 0, stop=di == ND - 1)
                yield
            vt = v_pool.tile([128, DVP], BF16, name=f"v{tb}", tag="v")
            nc.vector.tensor_copy(vt[:, 0:512], pa[:])
            pb = ps_proj.tile([128, DVP - 512], F32, name=f"vpb{tb}",
                              tag="pp")
            for di in range(ND):
                nc.tensor.matmul(pb[:], xv[di][:, tb * 128:(tb + 1) * 128],
                                 wv[di][:, 512:DVP],
                                 start=di == 0, stop=di == ND - 1)
                yield
            nc.vector.tensor_copy(vt[:, 512:DVP], pb[:])
            # ones columns (head slot 64) for the denominator trick
            v3 = vt[:].rearrange("p (h e) -> p h e", e=DH + 1)
            nc.vector.memset(v3[:, :, DH:DH + 1], 1.0)
            v_tiles[tb] = vt

        # ---- attention steps ----
        st_ctx = {}

        def att_state(p, strip):
            return st_ctx.setdefault((p, strip), {"et": {}, "grp": None,
                                                  "osd": None})

        def att_step(p, strip, tb):
            s = att_state(p, strip)
            sl = slice(strip * 512, strip * 512 + 512)
            sc = ps.tile([128, 1024], F32, name=f"sc{p}_{strip}_{tb}",
                         tag="ps")
            tsl = slice(tb * 128, (tb + 1) * 128)
            nc.tensor.matmul(sc[:, 0:512], kt_tiles[p][0:64, tsl],
                             qt_tiles[p][0:64, sl], start=True, stop=True)
            nc.tensor.matmul(sc[:, 512:1024], kt_tiles[p][64:128, tsl],
                             qt_tiles[p][64:128, sl], start=True, stop=True)
            et = e_pool.tile([128, 1024], BF16, name=f"e{p}_{strip}_{tb}",
                             tag="e")
            nc.scalar.activation(et[:], sc[:], Exp, scale=float(SCALE))
            s["et"][tb] = et

        def att_dmm(p, strip, tb):
            s = att_state(p, strip)
            if s["grp"] is None:
                s["grp"] = [ps_acc.tile([128, 260], F32,
                                        name=f"g{h}_{p}_{strip}", tag="acc")
                            for h in (0, 1)]
            c0 = p * 2 * (DH + 1)
            et = s["et"].pop(tb)
            vt = v_tiles[tb]
            for h in (0, 1):
                vsl = vt[:, c0 + h * (DH + 1):c0 + (h + 1) * (DH + 1)]
                for j in range(4):
                    nc.tensor.matmul(
                        s["grp"][h][:, j * 65:(j + 1) * 65],
                        et[:, h * 512 + j * 128:h * 512 + (j + 1) * 128],
                        vsl, start=tb == 0 and j == 0,
                        stop=tb == NT - 1 and j == 3,
                        skip_group_check=True)

        def att_norm(p, strip):
            s = att_state(p, strip)
            osd = osd_pool.tile([128, 512], BF16, name=f"osd{p}{strip}",
                                tag="osd")
            osd3 = osd[:].rearrange("p (j q) -> p j q", q=128)
            for h in (0, 1):
                rz = rz_pool.tile([128, 4], F32, name=f"rz{p}{strip}{h}",
                                  tag="rz")
                nc.vector.reciprocal_approx_fast(
                    rz[:], s["grp"][h][:, DH::DH + 1])
                g3 = s["grp"][h][:].rearrange("p (j e) -> p j e", e=DH + 1)
                nc.vector.tensor_mul(
                    osd3[:, :, h * DH:(h + 1) * DH], g3[:, :, 0:DH],
                    rz[:].unsqueeze(-1).broadcast_to((128, 4, DH)))
            s["osd"] = osd

        def att_transpose(p, strip):
            s = st_ctx.pop((p, strip))
            sl = slice(strip * 512, strip * 512 + 512)
            tr = ps_proj.tile([128, 512], BF16, name=f"tr{p}_{strip}",
                              tag="pp")
            for j in range(4):
                nc.tensor.transpose(tr[:, j * 128:(j + 1) * 128],
                                    s["osd"][:, j * 128:(j + 1) * 128],
                                    ident[:])
            nc.vector.tensor_copy(oht_tiles[p][:, sl], tr[:])

        oht_tiles = [
            oht_pool.tile([128, S], BF16, name=f"oht{p}", tag="oht")
            for p in range(NP)
        ]
        wo = []

        def gen_oproj(stt):
            ssl = slice(stt * 128, (stt + 1) * 128)
            o_t = o_pool.tile([128, D], BF16, name=f"o{stt}", tag="o")
            for hh, w in ((0, 512), (1, 256)):
                pp = ps_proj.tile([128, w], F32, name=f"opp{stt}_{hh}",
                                  tag="pp")
                csl = slice(hh * 512, hh * 512 + w)
                for di in range(ND):
                    nc.tensor.matmul(pp[:], oht_tiles[di][:, ssl],
                                     wo[di][:, csl],
                                     start=di == 0, stop=di == ND - 1)
                    yield
                nc.vector.tensor_add(o_t[:, csl], pp[:], bo_bc[:, csl])
                nc.sync.dma_start(out_d[ssl, csl], o_t[:, csl])

        def spawn_filler(p, strip):
            # projections spawn TWO units ahead of use so unit boundaries
            # never block in drive_until
            if strip == 0:
                if p == 0:
                    # interleave the V-proj units with the pair-2 trio so
                    # both complete early (batches force the vps; the trio
                    # rides along right behind the first vps)
                    for tb in range(NT):
                        spawn(f"vp{tb}", gen_vproj(tb))
                        if tb in (0, 1, 2):
                            spawn_qk_half(("q", "k", "k")[tb], 2,
                                          (0, 0, 1)[tb])
                elif p + 2 < NP:
                    spawn_qk_half("q", p + 2, 0)
                    spawn_qk_half("k", p + 2, 0)
                    spawn_qk_half("k", p + 2, 1)
                if p == 4:
                    spawn_qk_half("q", 0, 1)
                if p == 5:
                    spawn_qk_half("q", 1, 1)
                    for i in range(ND):
                        t = w_pool.tile([128, D], BF16, name=f"wot{i}",
                                        tag="w")
                        nc.sync.dma_start(
                            t[:], d["wot"][i * 128:(i + 1) * 128, :])
                        wo.append(t)
            else:
                if p + 2 < NP:
                    spawn_qk_half("q", p + 2, 1)
                # o-proj spawns trail the transpose pipeline by one unit:
                # oproj(k) reads all six strip-0 OHT tiles, the last of
                # which ((5,0)) is only transposed at unit 8's head
                if 2 <= p <= 5:
                    spawn(f"op{p - 2}", gen_oproj(p - 2))

        # ---- prologue: pair-0 h0 projections, then attention ----
        spawn_qk_half("k", 0, 0)
        spawn_qk_half("q", 0, 0)
        drive_all()
        spawn_qk_half("k", 0, 1)
        spawn_qk_half("q", 1, 0)
        spawn_qk_half("k", 1, 0)
        spawn_qk_half("k", 1, 1)

        # ---- exp-paced attention with a lagged dmm-batch FIFO: a unit's
        # attn@V batches drain during the following ~1.5 units, matching
        # when the V tiles' DMA+projection can actually complete ----
        seq = [(p, 0) for p in range(NP)] + [(p, 1) for p in range(NP)]
        pending = []          # FIFO of [p, strip, remaining tbs]
        tr_backlog = []       # normalized units awaiting PE transpose

        def pend_batch():
            if not pending:
                return
            ent = pending[0]
            tb = ent[2].pop(0)
            drive_until(f"vp{tb}")
            att_dmm(ent[0], ent[1], tb)
            if not ent[2]:
                att_norm(ent[0], ent[1])
                tr_backlog.append((ent[0], ent[1]))
                pending.pop(0)

        for idx, (p, strip) in enumerate(seq):
            drive_until(f"q{p}h{strip}")
            if strip == 0:
                # k{p}h1 is NOT forced here: scores tb<4 only read kt h0;
                # the tb-loop drives pull h1 through before tb=4 needs it
                drive_until(f"k{p}h0")
            att_step(p, strip, 0)
            if idx > 1:
                pend_batch()
            # transposes pop at the unit head so they always precede the
            # o-proj di=5 matmuls in the in-order PE queue
            if tr_backlog:
                att_transpose(*tr_backlog.pop(0))
            att_step(p, strip, 1)
            if idx > 1:
                pend_batch()
            if tr_backlog:
                att_transpose(*tr_backlog.pop(0))
            spawn_filler(p, strip)
            for tb in range(2, NT):
                drive(1)
                if tb == 4 and strip == 0:
                    # EMISSION-ORDER GUARD: scores tb>=4 read kt h1; the
                    # h1 generator must have emitted its write by now or
                    # the read semantically precedes the write (range
                    # semantics follow program order)
                    drive_until(f"k{p}h1")
                att_step(p, strip, tb)
                drive(2)
                if idx > 1 or (idx == 1 and tb >= 6):
                    pend_batch()
            pending.append([p, strip, list(range(NT))])

        # ---- epilogue ----
        drive_all()
        # finish the second-to-last unit's batches + transpose before the
        # tail o-proj generators reference its OHT
        while len(pending) > 1:
            pend_batch()
        while tr_backlog:
            att_transpose(*tr_backlog.pop(0))
        for stt in range(4, 8):
            spawn(f"op{stt}", gen_oproj(stt))
        # 5 safe yields (di 0..4 of first half) fill the final exp wait; a
        # 6th would emit a di=5 matmul ahead of the final transpose in the
        # in-order PE queue and deadlock on it
        drive(5)
        while pending:
            pend_batch()
        while tr_backlog:
            att_transpose(*tr_backlog.pop(0))
        drive_all()


def _prep(queries, keys, values, Wq, bq, Wk, bk, Wv, bv, Wo, bo):
    """Host-side prep: returns per-core input dicts."""
    wvt = np.asarray(Wv, np.float32).T              # (D, D) = (di, do)
    wvtp = np.zeros((D, DVP), np.float32)
    for h in range(H):
        wvtp[:, h * (DH + 1):h * (DH + 1) + DH] = \
            wvt[:, h * DH:(h + 1) * DH]
    bo_eff = (np.asarray(bo, np.float32)
              + np.asarray(Wo, np.float32) @ np.asarray(bv, np.float32))
    bf = lambda a: np.ascontiguousarray(np.asarray(a, np.float32)).astype(
        bfloat16)
    shared = {
        "wqt": bf(np.asarray(Wq, np.float32).T),
        "wkt": bf(np.asarray(Wk, np.float32).T),
        "wvtp": wvtp.astype(bfloat16),
        "wot": bf(np.asarray(Wo, np.float32).T),
        "ident": np.eye(128, dtype=np.float32).astype(bfloat16),
        "bqc": np.ascontiguousarray(
            np.asarray(bq, np.float32).reshape(ND, 128).T),
        "bkc": np.ascontiguousarray(
            np.asarray(bk, np.float32).reshape(ND, 128).T),
        "bor": np.ascontiguousarray(bo_eff.reshape(1, D)),
    }
    queries = np.asarray(queries, np.float32)
    keys = np.asarray(keys, np.float32)
    values = np.asarray(values, np.float32)
    in_maps = []
    for b in range(B):
        in_maps.append({
            "xqt": bf(queries[b].T),
            "xkt": bf(keys[b].T),
            "xvt": bf(values[b].T),
            **shared,
        })
    return in_maps


def _get_nc():
    if "nc" not in _CACHE:
        _CACHE["nc"] = _build_nc()
    return _CACHE["nc"]


def kernel(queries, keys, values, Wq, bq, Wk, bk, Wv, bv, Wo, bo):
    in_maps = _prep(queries, keys, values, Wq, bq, Wk, bk, Wv, bv, Wo, bo)
    nc = _get_nc()
    res = run_bass_kernel_spmd(nc, in_maps, core_ids=list(range(B)))
    return np.stack([res.results[b]["out"].astype(np.float32)
                     for b in range(B)], axis=0)
